# revision 30
# baseline (speedup 1.0000x reference)
"""Trainium2 Bass kernel for the dual-attention module (spatial + channel attention).

Contract: kernel(**inputs) takes the FULL inputs (x: (16,1024,64,64) f32 plus four
1x1-conv weight matrices) and returns the FULL output (16,1024,64,64) f32.
Internally shards data-parallel over batch across 8 NeuronCores (2 samples/core),
weights replicated.

Per-sample math (b, c=1024, ch=512, hw=4096):
  conv(w) = relu(w @ X)               X = x[b] as (1024, 4096)
  mask    = softmax(conv(w_qr))       over hw          (spatial attn branch)
  ctx     = conv(w_vr) @ mask         (ch,)
  s       = sigmoid(layernorm(ctx))   (ch,)
  avg     = softmax(mean_hw(conv(w_ql)))               (channel attn branch)
  chan    = sigmoid(avg @ conv(w_vl)) (hw,)
  out[0:512]    = x * (1 + s*chan)                     ("sequence")
  out[512:1024] = x * (1 + s + chan)                   ("parallel")

All four convs run in fp8e4m3 DoubleRow (weights pre-scaled x64; the 1/64 is
folded into later scalar passes).  The PE streams rhs rows at 2/cycle in DR so
every N=512 DR matmul costs ~235ns; per-core PE work is ~200us and everything
else must hide under it.

v2 changes vs the 272us baseline (trace-driven):
  - ONE ACT table set (exp_and_others: exp+tanh+relu+copy) resident for the
    whole kernel: sigmoids become 0.5+0.5*tanh(x/2) (exact identity, folded
    into the finale scalars with x pre-scaled by 0.5 on host), and
    rsqrt(var+eps) is computed on DVE via the int32 bit-trick + 2 Newton
    steps.  Kills ~23us of ACT_TABLE_LOAD thrash and the finalize stalls.
  - fp16 x and fp16 output (host converts back to f32): input DMA 16MB,
    output DMA 16MB instead of 16+32MB f32/bf16; finale DVE ops run on
    2-byte operands (full-rate stt, 2x tensor_scalar/tensor_tensor).
  - output dram layout is pair-major so each pair stores with ONE DMA
    trigger (the sync-engine DGE costs ~600ns/trigger); host transposes.
  - first xq pairs are loaded chunk-granular so the first matmul fires as
    soon as wqr + 512KB arrive (~11us instead of 14.7us).
  - sample-boundary barrier removed: pair-0 vl-conv matmuls are hoisted
    between finalizeA and finalizeB so the PE never drains at the
    layer-norm / chan-softmax reductions (fixes the HAM re-throttle at
    ~179us that left the whole tail at 1.2GHz).
  - last pair is chunk-serialized with matmuls emitted before the previous
    chunk's finale so the ACT queue never blocks the chan contraction.
"""

import sys

sys.path.insert(0, "/opt/trn_rl_repo")

import numpy as np

import concourse.bass as bass  # noqa: F401  (bass must import before bacc)
import concourse.tile as tile
from concourse import bacc, bass_isa, bass_utils, mybir

# Problem constants (hardcoded per contract).
B, C, H, W = 16, 1024, 64, 64
HW = H * W               # 4096
CH = C // 2              # 512
N_CORES = 8
S = B // N_CORES         # 2 samples per core
P = 128                  # SBUF partitions
KT = C // P              # 8 k-tiles over input channels
A2 = KT // 2             # 4 DoubleRow k-pair steps
MT = CH // P             # 4 m-tiles over output channels
NW = 512                 # n-chunk width (one PSUM bank of f32)
NCH = HW // NW           # 8 n-chunks
NP = NCH // 2            # 4 chunk-pairs
LN_EPS = 1e-5
WS = 64.0                # fp8 weight pre-scale

F32 = mybir.dt.float32
F32R = mybir.dt.float32r
F16 = mybir.dt.float16
U32 = mybir.dt.uint32
I32 = mybir.dt.int32
F8 = mybir.dt.float8e4
Alu = mybir.AluOpType
Act = mybir.ActivationFunctionType
AxX = mybir.AxisListType.X
DR = mybir.MatmulPerfMode.DoubleRow

_cache = {}


def _build():
    nc = bacc.Bacc(
        "TRN2",
        target_bir_lowering=False,
        debug=False,
        num_devices=N_CORES,
        dynamic_dma_scratch_size=512,
    )

    # pair-major layouts: one pair is a single DMA with contiguous bytes per
    # partition; weights partition-major.  out is pair-major too so a whole
    # pair stores with one DMA trigger.
    xq_d = nc.dram_tensor("xq", [S, NP, P, KT, 2, NW], F8, kind="ExternalInput")
    x_d = nc.dram_tensor("x", [S, NP, P, KT, 2 * NW], F16, kind="ExternalInput")
    wqr_d = nc.dram_tensor("wqr", [P, KT, P], F8, kind="ExternalInput")
    wvr_d = nc.dram_tensor("wvr", [P, KT, CH], F8, kind="ExternalInput")
    wql_d = nc.dram_tensor("wql", [P, KT, CH], F8, kind="ExternalInput")
    wvl_d = nc.dram_tensor("wvl", [P, KT, CH], F8, kind="ExternalInput")
    out_d = nc.dram_tensor("out", [S, NP, P, KT, 2 * NW], F16, kind="ExternalOutput")

    with tile.TileContext(nc) as tc:
        with (
            tc.tile_pool(name="xqp", bufs=5) as xqp,
            tc.tile_pool(name="xp", bufs=4) as xp,
            tc.tile_pool(name="wp", bufs=1) as wp,
            tc.tile_pool(name="okp", bufs=3) as okp,
            tc.tile_pool(name="etp", bufs=3) as etp,
            tc.tile_pool(name="chp", bufs=3) as chp,
            tc.tile_pool(name="atp", bufs=4) as atp,
            tc.tile_pool(name="deadp", bufs=1) as deadp,
            tc.tile_pool(name="thp", bufs=5) as thp,
            tc.tile_pool(name="smp", bufs=2) as smp,
            tc.tile_pool(name="erp", bufs=4) as erp,
            tc.tile_pool(name="psA", bufs=2, space="PSUM") as psA,
            tc.tile_pool(name="psB", bufs=6, space="PSUM") as psB,
        ):
            # ---- weight tiles ----
            wqr_sb = wp.tile([P, KT, P], F8, name="wqrsb", tag="wqrsb")
            wvr_sb = wp.tile([P, KT, CH], F8, name="wvrsb", tag="wvrsb")
            wql_sb = wp.tile([P, KT, CH], F8, name="wqlsb", tag="wqlsb")
            wvl_sb = wp.tile([P, KT, CH], F8, name="wvlsb", tag="wvlsb")
            wdma = {"wqr": wqr_d, "wvr": wvr_d, "wql": wql_d, "wvl": wvl_d}

            def load_w(t, nm):
                nc.sync.dma_start(t[:], wdma[nm].ap()[:])

            def emit_xq_load(s_, j_, split=False):
                t = xqp.tile([P, KT, 2, NW], F8, name=f"xq{s_}_{j_}", tag="xq")
                if split:
                    # chunk-granular so the first matmul gates on 512KB only
                    nc.sync.dma_start(t[:, :, 0, :], xq_d.ap()[s_, j_, :, :, 0, :])
                    nc.sync.dma_start(t[:, :, 1, :], xq_d.ap()[s_, j_, :, :, 1, :])
                else:
                    nc.sync.dma_start(t[:], xq_d.ap()[s_, j_])
                return t

            def emit_x_load(s_, j_):
                t = xp.tile([P, KT, 2 * NW], F16, name=f"x{s_}_{j_}", tag="x")
                nc.sync.dma_start(t[:], x_d.ap()[s_, j_])
                return t

            xq_all = {0: [], 1: []}
            x_all = {0: [], 1: []}
            load_w(wqr_sb, "wqr")
            t0 = xqp.tile([P, KT, 2, NW], F8, name="xq0_0", tag="xq")
            nc.sync.dma_start(t0[:, :, 0, :], xq_d.ap()[0, 0, :, :, 0, :])
            load_w(wvr_sb, "wvr")
            nc.sync.dma_start(t0[:, :, 1, :], xq_d.ap()[0, 0, :, :, 1, :])
            xq_all[0].append(t0)
            load_w(wql_sb, "wql")
            load_w(wvl_sb, "wvl")
            xq_all[0].append(emit_xq_load(0, 1))
            xq_all[0].append(emit_xq_load(0, 2))
            xq_all[0].append(emit_xq_load(0, 3))
            for j in range(NP):
                x_all[0].append(emit_x_load(0, j))

            # per-sample state
            st = {}

            def mk_state(s):
                st[s] = d = {}
                d["zpart"] = smp.tile([P, NCH], F32, name=f"zpart{s}", tag="zpart")
                d["ctxp"] = [
                    smp.tile([P, NCH], F32, name=f"ctxp{s}_{m}", tag=f"ctxp{m}")
                    for m in range(MT)
                ]
                d["gp"] = [
                    smp.tile([P, NCH], F32, name=f"gp{s}_{m}", tag=f"gp{m}")
                    for m in range(MT)
                ]
                d["th"] = {}

            def phaseA_chunk(s, j, q):
                """qr conv (mask logits) + vr conv (context) for chunk 2j+q."""
                d = st[s]
                i = 2 * j + q
                rhs = xq_all[s][j][:, :, q, :]
                psq = psA.tile([P, NW], F32, name=f"psq{s}_{i}", tag="psA")
                for a in range(A2):
                    nc.tensor.matmul(
                        psq[:],
                        wqr_sb[:, 2 * a : 2 * a + 2, :],
                        rhs[:, 2 * a : 2 * a + 2, :],
                        start=(a == 0), stop=(a == A2 - 1),
                        perf_mode=DR,
                    )
                # exp(relu(z)) == max(exp(z), 1): ACT exp (1/64 de-scales the
                # fp8 weight prescale), then DVE in-place max + Z accum
                et = etp.tile([P, NW], F32, name=f"et{s}_{i}", tag="et")
                nc.scalar.activation(et[:], psq[:], Act.Exp, scale=1.0 / WS)
                nc.vector.tensor_scalar(
                    et[:], et[:], 1.0, 0.0, Alu.max, Alu.add,
                    accum_out=d["zpart"][:, i : i + 1],
                )
                for m in range(MT):
                    psv = psB.tile([P, NW], F32, name=f"psv{s}a{i}_{m}", tag="psB")
                    for a in range(A2):
                        nc.tensor.matmul(
                            psv[:],
                            wvr_sb[:, 2 * a : 2 * a + 2, m * P : (m + 1) * P],
                            rhs[:, 2 * a : 2 * a + 2, :],
                            start=(a == 0), stop=(a == A2 - 1),
                            perf_mode=DR,
                        )
                    # ctx partial: sum_n relu(vr) * exp(relu(qr))
                    scr = deadp.tile([P, NW], F32, name=f"sttscr{s}", tag="sttscr")
                    nc.vector.scalar_tensor_tensor(
                        scr[:], psv[:], 0.0, et[:], Alu.max, Alu.mult,
                        accum_out=d["ctxp"][m][:, i : i + 1],
                    )

            def finalizeA(s):
                """mask Z + context -> layernorm stats + rstd (DVE-only rsqrt)."""
                d = st[s]
                Zt = smp.tile([P, 1], F32, name=f"Z{s}", tag="Z")
                nc.vector.tensor_reduce(Zt[:], d["zpart"][:], AxX, Alu.add)
                rZ = smp.tile([P, 1], F32, name=f"rZ{s}", tag="rZ")
                nc.vector.reciprocal(rZ[:], Zt[:])
                ctx44 = smp.tile([P, MT], F32, name=f"ctx44{s}", tag="ctx44")
                for m in range(MT):
                    cred = smp.tile([P, 1], F32, name=f"cred{s}_{m}", tag="cred")
                    nc.vector.tensor_reduce(cred[:], d["ctxp"][m][:], AxX, Alu.add)
                    # 1/64 restores the fp8 prescale: reference LN eps semantics
                    nc.vector.tensor_scalar(
                        ctx44[:, m : m + 1], cred[:], rZ[:], 1.0 / WS,
                        Alu.mult, Alu.mult,
                    )
                lnsum = smp.tile([P, MT], F32, name=f"lnsum{s}", tag="lnsum")
                nc.gpsimd.partition_all_reduce(
                    lnsum[:], ctx44[:], P, bass_isa.ReduceOp.add
                )
                tot = smp.tile([P, 1], F32, name=f"tot{s}", tag="tot")
                nc.vector.tensor_reduce(tot[:], lnsum[:], AxX, Alu.add)
                mu = smp.tile([P, 1], F32, name=f"mu{s}", tag="mu")
                nc.vector.tensor_scalar(mu[:], tot[:], 1.0 / CH, None, Alu.mult)
                d44 = smp.tile([P, MT], F32, name=f"d44{s}", tag="d44")
                nc.vector.tensor_scalar(d44[:], ctx44[:], mu[:], None, Alu.subtract)
                d2 = smp.tile([P, MT], F32, name=f"d2{s}", tag="d2")
                nc.vector.tensor_tensor(d2[:], d44[:], d44[:], Alu.mult)
                vsum = smp.tile([P, MT], F32, name=f"vsum{s}", tag="vsum")
                nc.gpsimd.partition_all_reduce(
                    vsum[:], d2[:], P, bass_isa.ReduceOp.add
                )
                veps = smp.tile([P, 1], F32, name=f"veps{s}", tag="veps")
                nc.vector.tensor_reduce(veps[:], vsum[:], AxX, Alu.add)
                # veps = var + eps
                nc.vector.tensor_scalar(
                    veps[:], veps[:], 1.0 / CH, LN_EPS, Alu.mult, Alu.add
                )
                # rstd = rsqrt(veps): int32 bit-trick seed + 2 Newton steps,
                # all on DVE ([P,1] ops) -- avoids the sqrt ACT table load.
                hbits = smp.tile([P, 1], U32, name=f"hb{s}", tag="hb")
                nc.vector.tensor_scalar(
                    hbits[:], veps[:].bitcast(U32), 1, None,
                    Alu.logical_shift_right,
                )
                # magic - h, computed as (-1)*h + magic in the int32 arith
                # path.  DVE int arithmetic routes through f32 (so the low
                # ~6 bits round) and uint add saturates -- the f32 rounding
                # only perturbs the Newton seed by ~1e-5 relative, fine.
                r0b = smp.tile([P, 1], I32, name=f"r0b{s}", tag="r0b")
                nc.vector.tensor_scalar(
                    r0b[:], hbits[:].bitcast(I32), -1, 0x5F3759DF,
                    Alu.mult, Alu.add,
                )
                y = r0b[:].bitcast(F32)
                for it in range(2):
                    y2 = smp.tile([P, 1], F32, name=f"ny{s}_{it}", tag=f"ny{it}")
                    nc.vector.tensor_tensor(y2[:], y, y, Alu.mult)
                    nc.vector.tensor_tensor(y2[:], y2[:], veps[:], Alu.mult)
                    nc.vector.tensor_scalar(
                        y2[:], y2[:], -0.5, 1.5, Alu.mult, Alu.add
                    )
                    nc.vector.tensor_tensor(y2[:], y2[:], y, Alu.mult)
                    y = y2[:]
                d["ctx44"], d["mu"], d["rstd"] = ctx44, mu, y

            def phaseB_chunk(s, j, q):
                """ql conv chunk; relu + mean partials, alternating engines."""
                d = st[s]
                i = 2 * j + q
                rhs = xq_all[s][j][:, :, q, :]
                for m in range(MT):
                    psv = psB.tile([P, NW], F32, name=f"psv{s}b{i}_{m}", tag="psB")
                    for a in range(A2):
                        nc.tensor.matmul(
                            psv[:],
                            wql_sb[:, 2 * a : 2 * a + 2, m * P : (m + 1) * P],
                            rhs[:, 2 * a : 2 * a + 2, :],
                            start=(a == 0), stop=(a == A2 - 1),
                            perf_mode=DR,
                        )
                    if m % 2 == 0:
                        scr = deadp.tile([P, NW], F32, name=f"qlscr{s}", tag="qlscr")
                        nc.scalar.activation(
                            scr[:], psv[:], Act.Relu,
                            accum_out=d["gp"][m][:, i : i + 1],
                        )
                    else:
                        scr2 = deadp.tile([P, NW], F32, name=f"sttscr{s}b", tag="sttscr")
                        nc.vector.tensor_scalar(
                            scr2[:], psv[:], 0.0, 0.0, Alu.max, Alu.add,
                            accum_out=d["gp"][m][:, i : i + 1],
                        )

            def finalizeB_pre(s):
                """chan-softmax weights e -> erep/rZc2.  Emitted BEFORE the
                hoisted pair-0 vl matmuls so the e44 exp runs on ACT ahead
                of the 8 th relus (else erep is ~5us late and the whole
                phase-C chain shifts)."""
                d = st[s]
                g44 = smp.tile([P, MT], F32, name=f"g44{s}", tag="g44")
                for m in range(MT):
                    nc.vector.tensor_reduce(
                        g44[:, m : m + 1], d["gp"][m][:], AxX, Alu.add
                    )
                e44 = smp.tile([P, MT], F32, name=f"e44{s}", tag="e44")
                nc.scalar.activation(e44[:], g44[:], Act.Exp, scale=1.0 / (HW * WS))
                ze = smp.tile([P, MT], F32, name=f"ze{s}", tag="ze")
                nc.gpsimd.partition_all_reduce(ze[:], e44[:], P, bass_isa.ReduceOp.add)
                zet = smp.tile([P, 1], F32, name=f"zet{s}", tag="zet")
                nc.vector.tensor_reduce(zet[:], ze[:], AxX, Alu.add)
                rZc = smp.tile([P, 1], F32, name=f"rZc{s}", tag="rZc")
                nc.vector.reciprocal(rZc[:], zet[:])
                # chan tanh scale = 0.5/(Z*64): the 1/64 de-scales the fp8
                # wvl prescale (erep itself is e44 unscaled in fp8)
                rZc2 = smp.tile([P, 1], F32, name=f"rZc2{s}", tag="rZc2")
                nc.vector.tensor_scalar(rZc2[:], rZc[:], 0.5 / WS, None, Alu.mult)
                erep = []
                for mp in range(2):
                    er = erp.tile([P, 2, P], F8, name=f"erep{s}_{mp}", tag="erep")
                    for i in range(2):
                        nc.vector.tensor_scalar(
                            er[:, i, :],
                            e44[:, 2 * mp + i : 2 * mp + i + 1].broadcast_to([P, P]),
                            1.0, None, Alu.mult,
                        )
                    erep.append(er)
                d["rZc2"], d["erep"] = rZc2, erep

            def finalizeB_post(s):
                """LN sigmoid (tanh form) -> finale scalars."""
                d = st[s]
                spre = smp.tile([P, MT], F32, name=f"spre{s}", tag="spre")
                nc.vector.tensor_scalar(
                    spre[:], d["ctx44"][:], d["mu"][:], d["rstd"],
                    Alu.subtract, Alu.mult,
                )
                # s = sigmoid(spre) = 0.5 + 0.5*tanh(spre/2); keep t44 = tanh
                t44 = smp.tile([P, MT], F32, name=f"t44{s}", tag="t44")
                nc.scalar.activation(t44[:], spre[:], Act.Tanh, scale=0.5)
                s44 = smp.tile([P, MT], F32, name=f"s44{s}", tag="s44")
                nc.vector.tensor_scalar(s44[:], t44[:], 0.5, 0.5, Alu.mult, Alu.add)
                # seq at-tile: at = s*tc + (2+s); with xh = x/2: out = at*xh
                b244 = smp.tile([P, MT], F32, name=f"b244{s}", tag="b244")
                nc.vector.tensor_scalar(b244[:], s44[:], 2.0, None, Alu.add)
                # par: out = (tc + (3+2s)) * xh = (tc + 4 + t) * xh
                ps44 = smp.tile([P, MT], F32, name=f"ps44{s}", tag="ps44")
                nc.vector.tensor_scalar(ps44[:], t44[:], 4.0, None, Alu.add)
                d["s44"], d["b244"], d["ps44"] = s44, b244, ps44

            def phaseCmm(s, j, q, last=False):
                """vl conv matmuls + relu for chunk (j,q); th tiles stored."""
                d = st[s]
                i = 2 * j + q
                rhs = xq_all[s][j][:, :, q, :]
                for m in range(MT):
                    psv = psB.tile([P, NW], F32, name=f"psv{s}c{i}_{m}", tag="psB")
                    for a in range(A2):
                        nc.tensor.matmul(
                            psv[:],
                            wvl_sb[:, 2 * a : 2 * a + 2, m * P : (m + 1) * P],
                            rhs[:, 2 * a : 2 * a + 2, :],
                            start=(a == 0), stop=(a == A2 - 1),
                            perf_mode=DR,
                        )
                    # th in fp8 (values are 64*theta <~ 230, inside e4m3
                    # range) packed as DoubleRow k-pairs for the contraction
                    if m % 2 == 0:
                        th = thp.tile([P, 2, NW], F8, name=f"th{s}_{i}_{m // 2}", tag="th")
                        d["th"][(i, m // 2)] = th
                    nc.scalar.activation(th[:, m % 2, :], psv[:], Act.Relu)

            def chan_chunk(s, j, q, chant):
                """e-weighted contraction + tanh for chunk q into chant."""
                d = st[s]
                i = 2 * j + q
                pschan = psA.tile([P, NW], F32, name=f"psc{s}_{i}", tag="psA")
                for mp in range(2):
                    nc.tensor.matmul(
                        pschan[:], d["erep"][mp][:], d["th"].pop((i, mp))[:],
                        start=(mp == 0), stop=(mp == 1),
                        perf_mode=DR, skip_group_check=True,
                    )
                # chant = tanh(0.5 * chan_logit); chan = 0.5 + 0.5*chant
                nc.scalar.activation(
                    chant[:, q * NW : (q + 1) * NW], pschan[:], Act.Tanh, scale=d["rZc2"][:]
                )

            def finale_pair(s, j, chant, tail=False):
                """pair-wide finale into one fp16 tile (store emitted by the
                caller so a waiting trigger never blocks the load queue).
                seq rows k<4:  out = (s*tc + 2+s) * xh
                par rows k>=4: out = (tc + 4+t) * xh          (xh = x/2)
                In the tail (s1 phase C) every engine must stay under the
                PE's ~8.2us/pair pace, so the at-construction is spread
                ACT/DVE/GpSimd; in the slot region ACT+DVE have slack."""
                d = st[s]
                s44, b244, ps44 = d["s44"], d["b244"], d["ps44"]
                xpair = x_all[s][j]
                okt = okp.tile([P, KT, 2 * NW], F16, name=f"ok{s}_{j}", tag="ok")
                for k in range(MT, KT):
                    nc.vector.scalar_tensor_tensor(
                        okt[:, k], chant[:], ps44[:, k - MT : k - MT + 1],
                        xpair[:, k], Alu.add, Alu.mult,
                    )
                for k in range(MT):
                    a_t = atp.tile([P, 2 * NW], F16, name=f"at{s}_{j}_{k}", tag="at")
                    if k == 0 or (k == 1 and tail != 1):
                        nc.scalar.activation(
                            a_t[:], chant[:], Act.Identity,
                            scale=s44[:, k : k + 1], bias=b244[:, k : k + 1],
                        )
                    elif k >= 2 and tail == 1:
                        # GpSimd at-tiles are slow (~2.2us) but free capacity
                        # -- only where a following pair of PE work hides them
                        nc.gpsimd.tensor_scalar(
                            a_t[:], chant[:], s44[:, k : k + 1],
                            b244[:, k : k + 1], Alu.mult, Alu.add,
                        )
                    else:
                        nc.vector.tensor_scalar(
                            a_t[:], chant[:], s44[:, k : k + 1],
                            b244[:, k : k + 1], Alu.mult, Alu.add,
                        )
                    nc.vector.tensor_tensor(
                        okt[:, k], a_t[:], xpair[:, k], Alu.mult
                    )
                return okt

            def store_pair(s, j, okt):
                nc.sync.dma_start(out_d.ap()[s, j], okt[:])

            def finale_last(s, j, chq):
                """last pair, chunk q=1 of pair NP-1: per-k stores so the
                end-of-kernel drain is pipelined."""
                d = st[s]
                s44, b244, ps44 = d["s44"], d["b244"], d["ps44"]
                xpair = x_all[s][j]
                okt = okp.tile([P, KT, 2 * NW], F16, name=f"okl{s}", tag="ok")
                for q in range(2):
                    for k in range(MT, KT):
                        nc.vector.scalar_tensor_tensor(
                            okt[:, k, q * NW : (q + 1) * NW], chq[q][:],
                            ps44[:, k - MT : k - MT + 1],
                            xpair[:, k, q * NW : (q + 1) * NW],
                            Alu.add, Alu.mult,
                        )
                    for k in range(MT):
                        a_t = atp.tile([P, NW], F16, name=f"atl{s}_{q}_{k}", tag="atl")
                        nc.vector.tensor_scalar(
                            a_t[:], chq[q][:], s44[:, k : k + 1],
                            b244[:, k : k + 1], Alu.mult, Alu.add,
                        )
                        nc.vector.tensor_tensor(
                            okt[:, k, q * NW : (q + 1) * NW], a_t[:],
                            xpair[:, k, q * NW : (q + 1) * NW], Alu.mult
                        )
                    # half-pair store on the sync queue (idle by now; one
                    # 1MB trigger beats 8 small gpsimd triggers)
                    nc.sync.dma_start(
                        out_d.ap()[s, j, :, :, q * NW : (q + 1) * NW],
                        okt[:, :, q * NW : (q + 1) * NW],
                    )

            # ================= schedule =================
            mk_state(0)
            for j in range(NP):
                for q in range(2):
                    phaseA_chunk(0, j, q)
            finalizeA(0)
            for j in range(NP):
                for q in range(2):
                    phaseB_chunk(0, j, q)
                if j == 1:
                    xq_all[1].append(emit_xq_load(1, 0))
            mk_state(1)
            # hoist pair-0 vl convs over finalizeB(0) so the PE stays busy
            # while DVE/ACT/GpSimd run the reductions
            finalizeB_pre(0)
            phaseCmm(0, 0, 0)
            phaseCmm(0, 0, 1)
            finalizeB_post(0)
            ch0 = chp.tile([P, 2 * NW], F16, name="ch0_0", tag="chant")
            chan_chunk(0, 0, 0, ch0)
            chan_chunk(0, 0, 1, ch0)
            # slots: s0 phase C pairs 1..3 interleaved with s1 phases A+B
            pend = [(0, ch0)]
            done = []
            cslots = [[1], [2], [3], []]
            for t in range(NP):
                # previous slot's store first: its okt is complete by now so
                # the trigger fires instantly and never blocks the loads below
                if done:
                    store_pair(0, *done.pop(0))
                if t < NP - 1:
                    xq_all[1].append(emit_xq_load(1, t + 1))
                for j in cslots[t]:
                    ch = chp.tile([P, 2 * NW], F16, name=f"ch0_{j}", tag="chant")
                    phaseCmm(0, j, 0)
                    chan_chunk(0, j, 0, ch)
                    phaseCmm(0, j, 1)
                    chan_chunk(0, j, 1, ch)
                    pend.append((j, ch))
                phaseA_chunk(1, t, 0)
                phaseA_chunk(1, t, 1)
                phaseB_chunk(1, t, 0)
                phaseB_chunk(1, t, 1)
                j, ch = pend.pop(0)
                done.append((j, finale_pair(0, j, ch)))
                x_all[1].append(emit_x_load(1, j))
            store_pair(0, *done.pop(0))
            finalizeA(1)
            finalizeB_pre(1)
            # hoist pair-0 vl convs over finalizeA/B(1): no PE drain at the
            # sample boundary
            phaseCmm(1, 0, 0)
            phaseCmm(1, 0, 1)
            finalizeB_post(1)
            ch = chp.tile([P, 2 * NW], F16, name="ch1_0", tag="chant")
            chan_chunk(1, 0, 0, ch)
            chan_chunk(1, 0, 1, ch)
            # pair 1 whole; finale(1,0) under its matmuls
            okt = finale_pair(1, 0, ch, tail=1)
            store_pair(1, 0, okt)
            ch2 = chp.tile([P, 2 * NW], F16, name="ch1_1", tag="chant")
            phaseCmm(1, 1, 0)
            chan_chunk(1, 1, 0, ch2)
            phaseCmm(1, 1, 1)
            chan_chunk(1, 1, 1, ch2)

            # pairs 2 and 3 fully chunk-granular: each chunk's chan tanh
            # lands right after its matmuls, and chunk finales trail by one
            # chunk of PE work instead of a whole pair, so only the final
            # chunk's finale (+1MB store) runs after the PE drains.
            okts = {}

            def chan_single(c, cq):
                d = st[1]
                pschan = psA.tile([P, NW], F32, name=f"pscs{c}", tag="psA")
                for mp in range(2):
                    nc.tensor.matmul(
                        pschan[:], d["erep"][mp][:], d["th"].pop((c, mp))[:],
                        start=(mp == 0), stop=(mp == 1),
                        perf_mode=DR, skip_group_check=True,
                    )
                nc.scalar.activation(cq[:], pschan[:], Act.Tanh, scale=d["rZc2"][:])

            def finale_chunk(c, cq):
                """chunk finale with per-engine split sized to the ~4.2us
                per-chunk PE pace: DVE 3 par stt + 2 at-ts + 4 tt; GpSimd 1
                par row (ts+tt) + 1 at; ACT 1 at."""
                j, q = c // 2, c % 2
                d = st[1]
                s44, b244, ps44 = d["s44"], d["b244"], d["ps44"]
                xpair = x_all[1][j]
                if q == 0:
                    okts[j] = okp.tile([P, KT, 2 * NW], F16, name=f"okc{j}", tag="ok")
                okt = okts[j]
                sl = slice(q * NW, (q + 1) * NW)
                for k in range(MT, KT - 1):
                    nc.vector.scalar_tensor_tensor(
                        okt[:, k, sl], cq[:], ps44[:, k - MT : k - MT + 1],
                        xpair[:, k, sl], Alu.add, Alu.mult,
                    )
                ap = atp.tile([P, NW], F16, name=f"app{c}", tag="atc")
                nc.gpsimd.tensor_scalar(
                    ap[:], cq[:], ps44[:, MT - 1 : MT], None, Alu.add
                )
                nc.gpsimd.tensor_tensor(
                    okt[:, KT - 1, sl], ap[:], xpair[:, KT - 1, sl], Alu.mult
                )
                for k in range(MT):
                    a_t = atp.tile([P, NW], F16, name=f"atc{c}_{k}", tag="atc")
                    if k == 0:
                        nc.scalar.activation(
                            a_t[:], cq[:], Act.Identity,
                            scale=s44[:, k : k + 1], bias=b244[:, k : k + 1],
                        )
                    elif k == 1:
                        nc.gpsimd.tensor_scalar(
                            a_t[:], cq[:], s44[:, k : k + 1],
                            b244[:, k : k + 1], Alu.mult, Alu.add,
                        )
                    else:
                        nc.vector.tensor_scalar(
                            a_t[:], cq[:], s44[:, k : k + 1],
                            b244[:, k : k + 1], Alu.mult, Alu.add,
                        )
                    nc.vector.tensor_tensor(
                        okt[:, k, sl], a_t[:], xpair[:, k, sl], Alu.mult
                    )
                nc.sync.dma_start(out_d.ap()[1, j, :, :, sl], okt[:, :, sl])

            chqs = {}
            for c in range(4, 8):
                j, q = c // 2, c % 2
                phaseCmm(1, j, q)
                cq = chp.tile([P, NW], F16, name=f"chq{c}", tag="chq")
                chan_single(c, cq)
                chqs[c] = cq
                if c == 4:
                    okt1 = finale_pair(1, 1, ch2, tail=1)
                    store_pair(1, 1, okt1)
                if c >= 6:
                    finale_chunk(c - 2, chqs.pop(c - 2))
            finale_chunk(6, chqs.pop(6))
            finale_chunk(7, chqs.pop(7))

    nc.compile()
    return nc


def _prep_inputs(x, w_qr, w_vr, w_ql, w_vl):
    import ml_dtypes

    f8 = np.dtype(ml_dtypes.float8_e4m3)
    x = np.asarray(x, dtype=np.float32).reshape(B, C, HW)
    wts = {}
    for nm, w in (("wvr", w_vr), ("wql", w_ql), ("wvl", w_vl)):
        w = np.asarray(w, dtype=np.float32)
        # (out, in) -> [P, KT, out]: wts[nm][p, k, o] = w[o, 128k + p]
        # scaled x64 into fp8 range (the 1/64 is folded back on-chip)
        wts[nm] = (
            np.ascontiguousarray(w.T.reshape(KT, P, CH).transpose(1, 0, 2)) * WS
        ).astype(f8)
    q = np.asarray(w_qr, dtype=np.float32).reshape(KT, P).T * WS  # [P, KT]
    wts["wqr"] = np.ascontiguousarray(
        np.broadcast_to(q[:, :, None], (P, KT, P))
    ).astype(f8)
    in_maps = []
    for c in range(N_CORES):
        m = dict(wts)
        # [S, pair, P, KT, 2, NW]: xf[s,j,p,k,q,n] = x[s, 128k+p, 512(2j+q)+n]
        xf = np.ascontiguousarray(
            x[S * c : S * (c + 1)]
            .reshape(S, KT, P, NP, 2, NW)
            .transpose(0, 3, 2, 1, 4, 5)
        )
        # finale consumes xh = x/2 in fp16 (tanh-form sigmoid identities)
        m["x"] = (xf * 0.5).astype(np.float16).reshape(S, NP, P, KT, 2 * NW)
        m["xq"] = xf.astype(f8)
        in_maps.append(m)
    return in_maps


def _run(x, w_qr, w_vr, w_ql, w_vl, trace=False):
    if "nc" not in _cache:
        _cache["nc"] = _build()
    nc = _cache["nc"]
    in_maps = _prep_inputs(x, w_qr, w_vr, w_ql, w_vl)
    res = bass_utils.run_bass_kernel_spmd(
        nc, in_maps, core_ids=list(range(N_CORES)), trace=trace
    )
    out = np.empty((B, C, HW), np.float32)
    for c in range(N_CORES):
        # [S, NP, P, KT, 2*NW] f16 -> [S, C, HW] f32
        r = res.results[c]["out"]
        out[S * c : S * (c + 1)] = (
            r.transpose(0, 3, 2, 1, 4).reshape(S, C, HW).astype(np.float32)
        )
    return out.reshape(B, C, H, W), res


def kernel(x, w_qr, w_vr, w_ql, w_vl):
    out, _ = _run(x, w_qr, w_vr, w_ql, w_vl, trace=False)
    return out


# revision 31
# speedup vs baseline: 1.3636x; 1.3636x over previous
"""Trainium2 Bass kernel for the dual-attention module (spatial + channel attention).

Contract: kernel(**inputs) takes the FULL inputs (x: (16,1024,64,64) f32 plus four
1x1-conv weight matrices) and returns the FULL output (16,1024,64,64) f32.
Internally shards data-parallel over batch across 8 NeuronCores (2 samples/core),
weights replicated.

Per-sample math (b, c=1024, ch=512, hw=4096):
  conv(w) = relu(w @ X)               X = x[b] as (1024, 4096)
  mask    = softmax(conv(w_qr))       over hw          (spatial attn branch)
  ctx     = conv(w_vr) @ mask         (ch,)
  s       = sigmoid(layernorm(ctx))   (ch,)
  avg     = softmax(mean_hw(conv(w_ql)))               (channel attn branch)
  chan    = sigmoid(avg @ conv(w_vl)) (hw,)
  out[0:512]    = x * (1 + s*chan)                     ("sequence")
  out[512:1024] = x * (1 + s + chan)                   ("parallel")

All four convs run in fp8e4m3 DoubleRow (weights pre-scaled x64; the 1/64 is
folded into later scalar passes).  The PE streams rhs rows at 2/cycle in DR so
every N=512 DR matmul costs ~235ns; per-core PE work is ~200us and everything
else must hide under it.

v2 changes vs the 272us baseline (trace-driven):
  - ONE ACT table set (exp_and_others: exp+tanh+relu+copy) resident for the
    whole kernel: sigmoids become 0.5+0.5*tanh(x/2) (exact identity, folded
    into the finale scalars with x pre-scaled by 0.5 on host), and
    rsqrt(var+eps) is computed on DVE via the int32 bit-trick + 2 Newton
    steps.  Kills ~23us of ACT_TABLE_LOAD thrash and the finalize stalls.
  - fp16 x and fp16 output (host converts back to f32): input DMA 16MB,
    output DMA 16MB instead of 16+32MB f32/bf16; finale DVE ops run on
    2-byte operands (full-rate stt, 2x tensor_scalar/tensor_tensor).
  - output dram layout is pair-major so each pair stores with ONE DMA
    trigger (the sync-engine DGE costs ~600ns/trigger); host transposes.
  - first xq pairs are loaded chunk-granular so the first matmul fires as
    soon as wqr + 512KB arrive (~11us instead of 14.7us).
  - sample-boundary barrier removed: pair-0 vl-conv matmuls are hoisted
    between finalizeA and finalizeB so the PE never drains at the
    layer-norm / chan-softmax reductions (fixes the HAM re-throttle at
    ~179us that left the whole tail at 1.2GHz).
  - last pair is chunk-serialized with matmuls emitted before the previous
    chunk's finale so the ACT queue never blocks the chan contraction.
"""

import sys

sys.path.insert(0, "/opt/trn_rl_repo")

import numpy as np

import concourse.bass as bass  # noqa: F401  (bass must import before bacc)
import concourse.tile as tile
from concourse import bacc, bass_isa, bass_utils, mybir

# Problem constants (hardcoded per contract).
B, C, H, W = 16, 1024, 64, 64
HW = H * W               # 4096
CH = C // 2              # 512
N_CORES = 8
S = B // N_CORES         # 2 samples per core
P = 128                  # SBUF partitions
KT = C // P              # 8 k-tiles over input channels
A2 = KT // 2             # 4 DoubleRow k-pair steps
MT = CH // P             # 4 m-tiles over output channels
NW = 512                 # n-chunk width (one PSUM bank of f32)
NCH = HW // NW           # 8 n-chunks
NP = NCH // 2            # 4 chunk-pairs
LN_EPS = 1e-5
WS = 64.0                # fp8 weight pre-scale

F32 = mybir.dt.float32
F32R = mybir.dt.float32r
F16 = mybir.dt.float16
U32 = mybir.dt.uint32
I32 = mybir.dt.int32
F8 = mybir.dt.float8e4
Alu = mybir.AluOpType
Act = mybir.ActivationFunctionType
AxX = mybir.AxisListType.X
DR = mybir.MatmulPerfMode.DoubleRow

_cache = {}


def _build():
    nc = bacc.Bacc(
        "TRN2",
        target_bir_lowering=False,
        debug=False,
        num_devices=N_CORES,
        dynamic_dma_scratch_size=512,
    )

    # pair-major layouts: one pair is a single DMA with contiguous bytes per
    # partition; weights partition-major.  out is pair-major too so a whole
    # pair stores with one DMA trigger.
    xq_d = nc.dram_tensor("xq", [S, NP, P, KT, 2, NW], F8, kind="ExternalInput")
    x_d = nc.dram_tensor("x", [S, NP, P, KT, 2 * NW], F16, kind="ExternalInput")
    wqr_d = nc.dram_tensor("wqr", [P, KT, P], F8, kind="ExternalInput")
    wvr_d = nc.dram_tensor("wvr", [P, KT, CH], F8, kind="ExternalInput")
    wql_d = nc.dram_tensor("wql", [P, KT, CH], F8, kind="ExternalInput")
    wvl_d = nc.dram_tensor("wvl", [P, KT, CH], F8, kind="ExternalInput")
    out_d = nc.dram_tensor("out", [S, NP, P, KT, 2 * NW], F16, kind="ExternalOutput")

    with tile.TileContext(nc) as tc:
        with (
            tc.tile_pool(name="xqp", bufs=5) as xqp,
            tc.tile_pool(name="xp", bufs=4) as xp,
            tc.tile_pool(name="wp", bufs=1) as wp,
            tc.tile_pool(name="okp", bufs=3) as okp,
            tc.tile_pool(name="etp", bufs=3) as etp,
            tc.tile_pool(name="chp", bufs=3) as chp,
            tc.tile_pool(name="atp", bufs=4) as atp,
            tc.tile_pool(name="deadp", bufs=1) as deadp,
            tc.tile_pool(name="thp", bufs=5) as thp,
            tc.tile_pool(name="smp", bufs=2) as smp,
            tc.tile_pool(name="erp", bufs=4) as erp,
            tc.tile_pool(name="psA", bufs=2, space="PSUM") as psA,
            tc.tile_pool(name="psB", bufs=6, space="PSUM") as psB,
        ):
            # ---- weight tiles ----
            wqr_sb = wp.tile([P, KT, P], F8, name="wqrsb", tag="wqrsb")
            wvr_sb = wp.tile([P, KT, CH], F8, name="wvrsb", tag="wvrsb")
            wql_sb = wp.tile([P, KT, CH], F8, name="wqlsb", tag="wqlsb")
            wvl_sb = wp.tile([P, KT, CH], F8, name="wvlsb", tag="wvlsb")
            wdma = {"wqr": wqr_d, "wvr": wvr_d, "wql": wql_d, "wvl": wvl_d}

            def load_w(t, nm):
                nc.sync.dma_start(t[:], wdma[nm].ap()[:])

            def emit_xq_load(s_, j_, split=False):
                t = xqp.tile([P, KT, 2, NW], F8, name=f"xq{s_}_{j_}", tag="xq")
                if split:
                    # chunk-granular so the first matmul gates on 512KB only
                    nc.sync.dma_start(t[:, :, 0, :], xq_d.ap()[s_, j_, :, :, 0, :])
                    nc.sync.dma_start(t[:, :, 1, :], xq_d.ap()[s_, j_, :, :, 1, :])
                else:
                    nc.sync.dma_start(t[:], xq_d.ap()[s_, j_])
                return t

            def emit_x_load(s_, j_):
                t = xp.tile([P, KT, 2 * NW], F16, name=f"x{s_}_{j_}", tag="x")
                nc.sync.dma_start(t[:], x_d.ap()[s_, j_])
                return t

            xq_all = {0: [], 1: []}
            x_all = {0: [], 1: []}
            load_w(wqr_sb, "wqr")
            t0 = xqp.tile([P, KT, 2, NW], F8, name="xq0_0", tag="xq")
            nc.sync.dma_start(t0[:, :, 0, :], xq_d.ap()[0, 0, :, :, 0, :])
            load_w(wvr_sb, "wvr")
            nc.sync.dma_start(t0[:, :, 1, :], xq_d.ap()[0, 0, :, :, 1, :])
            xq_all[0].append(t0)
            load_w(wql_sb, "wql")
            load_w(wvl_sb, "wvl")
            xq_all[0].append(emit_xq_load(0, 1))
            xq_all[0].append(emit_xq_load(0, 2))
            xq_all[0].append(emit_xq_load(0, 3))
            for j in range(NP):
                x_all[0].append(emit_x_load(0, j))

            # per-sample state
            st = {}

            def mk_state(s):
                st[s] = d = {}
                d["zpart"] = smp.tile([P, NCH], F32, name=f"zpart{s}", tag="zpart")
                d["ctxp"] = [
                    smp.tile([P, NCH], F32, name=f"ctxp{s}_{m}", tag=f"ctxp{m}")
                    for m in range(MT)
                ]
                d["gp"] = [
                    smp.tile([P, NCH], F32, name=f"gp{s}_{m}", tag=f"gp{m}")
                    for m in range(MT)
                ]
                d["th"] = {}

            def phaseA_chunk(s, j, q):
                """qr conv (mask logits) + vr conv (context) for chunk 2j+q."""
                d = st[s]
                i = 2 * j + q
                rhs = xq_all[s][j][:, :, q, :]
                psq = psA.tile([P, NW], F32, name=f"psq{s}_{i}", tag="psA")
                for a in range(A2):
                    nc.tensor.matmul(
                        psq[:],
                        wqr_sb[:, 2 * a : 2 * a + 2, :],
                        rhs[:, 2 * a : 2 * a + 2, :],
                        start=(a == 0), stop=(a == A2 - 1),
                        perf_mode=DR,
                    )
                # exp(relu(z)) == max(exp(z), 1): ACT exp (1/64 de-scales the
                # fp8 weight prescale), then DVE in-place max + Z accum
                et = etp.tile([P, NW], F32, name=f"et{s}_{i}", tag="et")
                nc.scalar.activation(et[:], psq[:], Act.Exp, scale=1.0 / WS)
                nc.vector.tensor_scalar(
                    et[:], et[:], 1.0, 0.0, Alu.max, Alu.add,
                    accum_out=d["zpart"][:, i : i + 1],
                )
                for m in range(MT):
                    psv = psB.tile([P, NW], F32, name=f"psv{s}a{i}_{m}", tag="psB")
                    for a in range(A2):
                        nc.tensor.matmul(
                            psv[:],
                            wvr_sb[:, 2 * a : 2 * a + 2, m * P : (m + 1) * P],
                            rhs[:, 2 * a : 2 * a + 2, :],
                            start=(a == 0), stop=(a == A2 - 1),
                            perf_mode=DR,
                        )
                    # ctx partial: sum_n relu(vr) * exp(relu(qr))
                    scr = deadp.tile([P, NW], F32, name=f"sttscr{s}", tag="sttscr")
                    nc.vector.scalar_tensor_tensor(
                        scr[:], psv[:], 0.0, et[:], Alu.max, Alu.mult,
                        accum_out=d["ctxp"][m][:, i : i + 1],
                    )

            def finalizeA(s):
                """mask Z + context -> layernorm stats + rstd (DVE-only rsqrt)."""
                d = st[s]
                Zt = smp.tile([P, 1], F32, name=f"Z{s}", tag="Z")
                nc.vector.tensor_reduce(Zt[:], d["zpart"][:], AxX, Alu.add)
                rZ = smp.tile([P, 1], F32, name=f"rZ{s}", tag="rZ")
                nc.vector.reciprocal(rZ[:], Zt[:])
                ctx44 = smp.tile([P, MT], F32, name=f"ctx44{s}", tag="ctx44")
                for m in range(MT):
                    cred = smp.tile([P, 1], F32, name=f"cred{s}_{m}", tag="cred")
                    nc.vector.tensor_reduce(cred[:], d["ctxp"][m][:], AxX, Alu.add)
                    # 1/64 restores the fp8 prescale: reference LN eps semantics
                    nc.vector.tensor_scalar(
                        ctx44[:, m : m + 1], cred[:], rZ[:], 1.0 / WS,
                        Alu.mult, Alu.mult,
                    )
                lnsum = smp.tile([P, MT], F32, name=f"lnsum{s}", tag="lnsum")
                nc.gpsimd.partition_all_reduce(
                    lnsum[:], ctx44[:], P, bass_isa.ReduceOp.add
                )
                tot = smp.tile([P, 1], F32, name=f"tot{s}", tag="tot")
                nc.vector.tensor_reduce(tot[:], lnsum[:], AxX, Alu.add)
                mu = smp.tile([P, 1], F32, name=f"mu{s}", tag="mu")
                nc.vector.tensor_scalar(mu[:], tot[:], 1.0 / CH, None, Alu.mult)
                d44 = smp.tile([P, MT], F32, name=f"d44{s}", tag="d44")
                nc.vector.tensor_scalar(d44[:], ctx44[:], mu[:], None, Alu.subtract)
                d2 = smp.tile([P, MT], F32, name=f"d2{s}", tag="d2")
                nc.vector.tensor_tensor(d2[:], d44[:], d44[:], Alu.mult)
                vsum = smp.tile([P, MT], F32, name=f"vsum{s}", tag="vsum")
                nc.gpsimd.partition_all_reduce(
                    vsum[:], d2[:], P, bass_isa.ReduceOp.add
                )
                veps = smp.tile([P, 1], F32, name=f"veps{s}", tag="veps")
                nc.vector.tensor_reduce(veps[:], vsum[:], AxX, Alu.add)
                # veps = var + eps
                nc.vector.tensor_scalar(
                    veps[:], veps[:], 1.0 / CH, LN_EPS, Alu.mult, Alu.add
                )
                # rstd = rsqrt(veps): int32 bit-trick seed + 2 Newton steps,
                # all on DVE ([P,1] ops) -- avoids the sqrt ACT table load.
                hbits = smp.tile([P, 1], U32, name=f"hb{s}", tag="hb")
                nc.vector.tensor_scalar(
                    hbits[:], veps[:].bitcast(U32), 1, None,
                    Alu.logical_shift_right,
                )
                # magic - h, computed as (-1)*h + magic in the int32 arith
                # path.  DVE int arithmetic routes through f32 (so the low
                # ~6 bits round) and uint add saturates -- the f32 rounding
                # only perturbs the Newton seed by ~1e-5 relative, fine.
                r0b = smp.tile([P, 1], I32, name=f"r0b{s}", tag="r0b")
                nc.vector.tensor_scalar(
                    r0b[:], hbits[:].bitcast(I32), -1, 0x5F3759DF,
                    Alu.mult, Alu.add,
                )
                y = r0b[:].bitcast(F32)
                for it in range(2):
                    y2 = smp.tile([P, 1], F32, name=f"ny{s}_{it}", tag=f"ny{it}")
                    nc.vector.tensor_tensor(y2[:], y, y, Alu.mult)
                    nc.vector.tensor_tensor(y2[:], y2[:], veps[:], Alu.mult)
                    nc.vector.tensor_scalar(
                        y2[:], y2[:], -0.5, 1.5, Alu.mult, Alu.add
                    )
                    nc.vector.tensor_tensor(y2[:], y2[:], y, Alu.mult)
                    y = y2[:]
                d["ctx44"], d["mu"], d["rstd"] = ctx44, mu, y

            def phaseB_chunk(s, j, q):
                """ql conv chunk; relu + mean partials, alternating engines."""
                d = st[s]
                i = 2 * j + q
                rhs = xq_all[s][j][:, :, q, :]
                for m in range(MT):
                    psv = psB.tile([P, NW], F32, name=f"psv{s}b{i}_{m}", tag="psB")
                    for a in range(A2):
                        nc.tensor.matmul(
                            psv[:],
                            wql_sb[:, 2 * a : 2 * a + 2, m * P : (m + 1) * P],
                            rhs[:, 2 * a : 2 * a + 2, :],
                            start=(a == 0), stop=(a == A2 - 1),
                            perf_mode=DR,
                        )
                    if m % 2 == 0:
                        scr = deadp.tile([P, NW], F32, name=f"qlscr{s}", tag="qlscr")
                        nc.scalar.activation(
                            scr[:], psv[:], Act.Relu,
                            accum_out=d["gp"][m][:, i : i + 1],
                        )
                    else:
                        scr2 = deadp.tile([P, NW], F32, name=f"sttscr{s}b", tag="sttscr")
                        nc.vector.tensor_scalar(
                            scr2[:], psv[:], 0.0, 0.0, Alu.max, Alu.add,
                            accum_out=d["gp"][m][:, i : i + 1],
                        )

            def finalizeB_pre(s):
                """chan-softmax weights e -> erep/rZc2.  Emitted BEFORE the
                hoisted pair-0 vl matmuls so the e44 exp runs on ACT ahead
                of the 8 th relus (else erep is ~5us late and the whole
                phase-C chain shifts)."""
                d = st[s]
                g44 = smp.tile([P, MT], F32, name=f"g44{s}", tag="g44")
                for m in range(MT):
                    nc.vector.tensor_reduce(
                        g44[:, m : m + 1], d["gp"][m][:], AxX, Alu.add
                    )
                e44 = smp.tile([P, MT], F32, name=f"e44{s}", tag="e44")
                nc.scalar.activation(e44[:], g44[:], Act.Exp, scale=1.0 / (HW * WS))
                ze = smp.tile([P, MT], F32, name=f"ze{s}", tag="ze")
                nc.gpsimd.partition_all_reduce(ze[:], e44[:], P, bass_isa.ReduceOp.add)
                zet = smp.tile([P, 1], F32, name=f"zet{s}", tag="zet")
                nc.vector.tensor_reduce(zet[:], ze[:], AxX, Alu.add)
                rZc = smp.tile([P, 1], F32, name=f"rZc{s}", tag="rZc")
                nc.vector.reciprocal(rZc[:], zet[:])
                # chan tanh scale = 0.5/(Z*64): the 1/64 de-scales the fp8
                # wvl prescale (erep itself is e44 unscaled in fp8)
                rZc2 = smp.tile([P, 1], F32, name=f"rZc2{s}", tag="rZc2")
                nc.vector.tensor_scalar(rZc2[:], rZc[:], 0.5 / WS, None, Alu.mult)
                erep = []
                for mp in range(2):
                    er = erp.tile([P, 2, P], F8, name=f"erep{s}_{mp}", tag="erep")
                    for i in range(2):
                        nc.vector.tensor_scalar(
                            er[:, i, :],
                            e44[:, 2 * mp + i : 2 * mp + i + 1].broadcast_to([P, P]),
                            1.0, None, Alu.mult,
                        )
                    erep.append(er)
                d["rZc2"], d["erep"] = rZc2, erep

            def finalizeB_post(s):
                """LN sigmoid (tanh form) -> finale scalars."""
                d = st[s]
                spre = smp.tile([P, MT], F32, name=f"spre{s}", tag="spre")
                nc.vector.tensor_scalar(
                    spre[:], d["ctx44"][:], d["mu"][:], d["rstd"],
                    Alu.subtract, Alu.mult,
                )
                # s = sigmoid(spre) = 0.5 + 0.5*tanh(spre/2); keep t44 = tanh
                t44 = smp.tile([P, MT], F32, name=f"t44{s}", tag="t44")
                nc.scalar.activation(t44[:], spre[:], Act.Tanh, scale=0.5)
                s44 = smp.tile([P, MT], F32, name=f"s44{s}", tag="s44")
                nc.vector.tensor_scalar(s44[:], t44[:], 0.5, 0.5, Alu.mult, Alu.add)
                # seq at-tile: at = s*tc + (2+s); with xh = x/2: out = at*xh
                b244 = smp.tile([P, MT], F32, name=f"b244{s}", tag="b244")
                nc.vector.tensor_scalar(b244[:], s44[:], 2.0, None, Alu.add)
                # par: out = (tc + (3+2s)) * xh = (tc + 4 + t) * xh
                ps44 = smp.tile([P, MT], F32, name=f"ps44{s}", tag="ps44")
                nc.vector.tensor_scalar(ps44[:], t44[:], 4.0, None, Alu.add)
                d["s44"], d["b244"], d["ps44"] = s44, b244, ps44

            def phaseCmm(s, j, q, last=False):
                """vl conv matmuls + relu for chunk (j,q); th tiles stored."""
                d = st[s]
                i = 2 * j + q
                rhs = xq_all[s][j][:, :, q, :]
                for m in range(MT):
                    psv = psB.tile([P, NW], F32, name=f"psv{s}c{i}_{m}", tag="psB")
                    for a in range(A2):
                        nc.tensor.matmul(
                            psv[:],
                            wvl_sb[:, 2 * a : 2 * a + 2, m * P : (m + 1) * P],
                            rhs[:, 2 * a : 2 * a + 2, :],
                            start=(a == 0), stop=(a == A2 - 1),
                            perf_mode=DR,
                        )
                    # th in fp8 (values are 64*theta <~ 230, inside e4m3
                    # range) packed as DoubleRow k-pairs for the contraction
                    if m % 2 == 0:
                        th = thp.tile([P, 2, NW], F8, name=f"th{s}_{i}_{m // 2}", tag="th")
                        d["th"][(i, m // 2)] = th
                    nc.scalar.activation(th[:, m % 2, :], psv[:], Act.Relu)

            def chan_chunk(s, j, q, chant):
                """e-weighted contraction + tanh for chunk q into chant."""
                d = st[s]
                i = 2 * j + q
                pschan = psA.tile([P, NW], F32, name=f"psc{s}_{i}", tag="psA")
                for mp in range(2):
                    nc.tensor.matmul(
                        pschan[:], d["erep"][mp][:], d["th"].pop((i, mp))[:],
                        start=(mp == 0), stop=(mp == 1),
                        perf_mode=DR, skip_group_check=True,
                    )
                # chant = tanh(0.5 * chan_logit); chan = 0.5 + 0.5*chant
                nc.scalar.activation(
                    chant[:, q * NW : (q + 1) * NW], pschan[:], Act.Tanh, scale=d["rZc2"][:]
                )

            def finale_pair(s, j, chant, tail=False):
                """pair-wide finale into one fp16 tile (store emitted by the
                caller so a waiting trigger never blocks the load queue).
                seq rows k<4:  out = (s*tc + 2+s) * xh
                par rows k>=4: out = (tc + 4+t) * xh          (xh = x/2)
                In the tail (s1 phase C) every engine must stay under the
                PE's ~8.2us/pair pace, so the at-construction is spread
                ACT/DVE/GpSimd; in the slot region ACT+DVE have slack."""
                d = st[s]
                s44, b244, ps44 = d["s44"], d["b244"], d["ps44"]
                xpair = x_all[s][j]
                okt = okp.tile([P, KT, 2 * NW], F16, name=f"ok{s}_{j}", tag="ok")
                for k in range(MT, KT):
                    nc.vector.scalar_tensor_tensor(
                        okt[:, k], chant[:], ps44[:, k - MT : k - MT + 1],
                        xpair[:, k], Alu.add, Alu.mult,
                    )
                for k in range(MT):
                    a_t = atp.tile([P, 2 * NW], F16, name=f"at{s}_{j}_{k}", tag="at")
                    if k == 0 or (k == 1 and tail != 1):
                        nc.scalar.activation(
                            a_t[:], chant[:], Act.Identity,
                            scale=s44[:, k : k + 1], bias=b244[:, k : k + 1],
                        )
                    elif k >= 2 and tail == 1:
                        # GpSimd at-tiles are slow (~2.2us) but free capacity
                        # -- only where a following pair of PE work hides them
                        nc.gpsimd.tensor_scalar(
                            a_t[:], chant[:], s44[:, k : k + 1],
                            b244[:, k : k + 1], Alu.mult, Alu.add,
                        )
                    else:
                        nc.vector.tensor_scalar(
                            a_t[:], chant[:], s44[:, k : k + 1],
                            b244[:, k : k + 1], Alu.mult, Alu.add,
                        )
                    nc.vector.tensor_tensor(
                        okt[:, k], a_t[:], xpair[:, k], Alu.mult
                    )
                return okt

            def store_pair(s, j, okt):
                nc.sync.dma_start(out_d.ap()[s, j], okt[:])

            def finale_last(s, j, chq):
                """last pair, chunk q=1 of pair NP-1: per-k stores so the
                end-of-kernel drain is pipelined."""
                d = st[s]
                s44, b244, ps44 = d["s44"], d["b244"], d["ps44"]
                xpair = x_all[s][j]
                okt = okp.tile([P, KT, 2 * NW], F16, name=f"okl{s}", tag="ok")
                for q in range(2):
                    for k in range(MT, KT):
                        nc.vector.scalar_tensor_tensor(
                            okt[:, k, q * NW : (q + 1) * NW], chq[q][:],
                            ps44[:, k - MT : k - MT + 1],
                            xpair[:, k, q * NW : (q + 1) * NW],
                            Alu.add, Alu.mult,
                        )
                    for k in range(MT):
                        a_t = atp.tile([P, NW], F16, name=f"atl{s}_{q}_{k}", tag="atl")
                        nc.vector.tensor_scalar(
                            a_t[:], chq[q][:], s44[:, k : k + 1],
                            b244[:, k : k + 1], Alu.mult, Alu.add,
                        )
                        nc.vector.tensor_tensor(
                            okt[:, k, q * NW : (q + 1) * NW], a_t[:],
                            xpair[:, k, q * NW : (q + 1) * NW], Alu.mult
                        )
                    # half-pair store on the sync queue (idle by now; one
                    # 1MB trigger beats 8 small gpsimd triggers)
                    nc.sync.dma_start(
                        out_d.ap()[s, j, :, :, q * NW : (q + 1) * NW],
                        okt[:, :, q * NW : (q + 1) * NW],
                    )

            # ================= schedule =================
            mk_state(0)
            for j in range(NP):
                for q in range(2):
                    phaseA_chunk(0, j, q)
            finalizeA(0)
            for j in range(NP):
                for q in range(2):
                    phaseB_chunk(0, j, q)
                if j == 1:
                    xq_all[1].append(emit_xq_load(1, 0))
            mk_state(1)
            # hoist pair-0 vl convs over finalizeB(0) so the PE stays busy
            # while DVE/ACT/GpSimd run the reductions
            finalizeB_pre(0)
            phaseCmm(0, 0, 0)
            phaseCmm(0, 0, 1)
            finalizeB_post(0)
            ch0 = chp.tile([P, 2 * NW], F16, name="ch0_0", tag="chant")
            chan_chunk(0, 0, 0, ch0)
            chan_chunk(0, 0, 1, ch0)
            # slots: s0 phase C pairs 1..3 interleaved with s1 phases A+B
            pend = [(0, ch0)]
            done = []
            cslots = [[1], [2], [3], []]
            for t in range(NP):
                # previous slot's store first: its okt is complete by now so
                # the trigger fires instantly and never blocks the loads below
                if done:
                    store_pair(0, *done.pop(0))
                if t < NP - 1:
                    xq_all[1].append(emit_xq_load(1, t + 1))
                for j in cslots[t]:
                    ch = chp.tile([P, 2 * NW], F16, name=f"ch0_{j}", tag="chant")
                    phaseCmm(0, j, 0)
                    chan_chunk(0, j, 0, ch)
                    phaseCmm(0, j, 1)
                    chan_chunk(0, j, 1, ch)
                    pend.append((j, ch))
                phaseA_chunk(1, t, 0)
                phaseA_chunk(1, t, 1)
                phaseB_chunk(1, t, 0)
                phaseB_chunk(1, t, 1)
                j, ch = pend.pop(0)
                done.append((j, finale_pair(0, j, ch)))
                x_all[1].append(emit_x_load(1, j))
            store_pair(0, *done.pop(0))
            finalizeA(1)
            finalizeB_pre(1)
            # hoist pair-0 vl convs over finalizeA/B(1): no PE drain at the
            # sample boundary
            phaseCmm(1, 0, 0)
            phaseCmm(1, 0, 1)
            finalizeB_post(1)
            ch = chp.tile([P, 2 * NW], F16, name="ch1_0", tag="chant")
            chan_chunk(1, 0, 0, ch)
            chan_chunk(1, 0, 1, ch)
            prev = (0, ch)
            for j in range(1, NP - 1):
                okt = finale_pair(1, prev[0], prev[1], tail=1)
                store_pair(1, prev[0], okt)
                ch2 = chp.tile([P, 2 * NW], F16, name=f"ch1_{j}", tag="chant")
                phaseCmm(1, j, 0)
                chan_chunk(1, j, 0, ch2)
                phaseCmm(1, j, 1)
                chan_chunk(1, j, 1, ch2)
                prev = (j, ch2)
            okt = finale_pair(1, prev[0], prev[1], tail=2)
            store_pair(1, prev[0], okt)
            # last pair: chunk-serialized
            chq = []
            for q in range(2):
                phaseCmm(1, NP - 1, q, last=True)
                c1 = chp.tile([P, NW], F16, name=f"chl{q}", tag="chant")
                pschan = psA.tile([P, NW], F32, name=f"pscl{q}", tag="psA")
                d = st[1]
                for mp in range(2):
                    nc.tensor.matmul(
                        pschan[:], d["erep"][mp][:],
                        d["th"].pop((2 * (NP - 1) + q, mp))[:],
                        start=(mp == 0), stop=(mp == 1),
                        perf_mode=DR, skip_group_check=True,
                    )
                nc.scalar.activation(
                    c1[:], pschan[:], Act.Tanh, scale=d["rZc2"][:]
                )
                chq.append(c1)
            finale_last(1, NP - 1, chq)

    nc.compile()
    return nc


def _prep_inputs(x, w_qr, w_vr, w_ql, w_vl):
    import ml_dtypes

    f8 = np.dtype(ml_dtypes.float8_e4m3)
    x = np.asarray(x, dtype=np.float32).reshape(B, C, HW)
    wts = {}
    for nm, w in (("wvr", w_vr), ("wql", w_ql), ("wvl", w_vl)):
        w = np.asarray(w, dtype=np.float32)
        # (out, in) -> [P, KT, out]: wts[nm][p, k, o] = w[o, 128k + p]
        # scaled x64 into fp8 range (the 1/64 is folded back on-chip)
        wts[nm] = (
            np.ascontiguousarray(w.T.reshape(KT, P, CH).transpose(1, 0, 2)) * WS
        ).astype(f8)
    q = np.asarray(w_qr, dtype=np.float32).reshape(KT, P).T * WS  # [P, KT]
    wts["wqr"] = np.ascontiguousarray(
        np.broadcast_to(q[:, :, None], (P, KT, P))
    ).astype(f8)
    in_maps = []
    for c in range(N_CORES):
        m = dict(wts)
        # [S, pair, P, KT, 2, NW]: xf[s,j,p,k,q,n] = x[s, 128k+p, 512(2j+q)+n]
        xf = np.ascontiguousarray(
            x[S * c : S * (c + 1)]
            .reshape(S, KT, P, NP, 2, NW)
            .transpose(0, 3, 2, 1, 4, 5)
        )
        # finale consumes xh = x/2 in fp16 (tanh-form sigmoid identities)
        m["x"] = (xf * 0.5).astype(np.float16).reshape(S, NP, P, KT, 2 * NW)
        m["xq"] = xf.astype(f8)
        in_maps.append(m)
    return in_maps


def _run(x, w_qr, w_vr, w_ql, w_vl, trace=False):
    if "nc" not in _cache:
        _cache["nc"] = _build()
    nc = _cache["nc"]
    in_maps = _prep_inputs(x, w_qr, w_vr, w_ql, w_vl)
    res = bass_utils.run_bass_kernel_spmd(
        nc, in_maps, core_ids=list(range(N_CORES)), trace=trace
    )
    out = np.empty((B, C, HW), np.float32)
    for c in range(N_CORES):
        # [S, NP, P, KT, 2*NW] f16 -> [S, C, HW] f32
        r = res.results[c]["out"]
        out[S * c : S * (c + 1)] = (
            r.transpose(0, 3, 2, 1, 4).reshape(S, C, HW).astype(np.float32)
        )
    return out.reshape(B, C, H, W), res


def kernel(x, w_qr, w_vr, w_ql, w_vl):
    out, _ = _run(x, w_qr, w_vr, w_ql, w_vl, trace=False)
    return out
